# revision 7
# baseline (speedup 1.0000x reference)
"""Trainium2 Bass kernel for nn_MemLayer_7275674600019 (retrieval_knn).

Math: the reference computes
    queries = (x @ Wq.T)                       [B, H, Q]
    attn    = softmax(queries @ keys.T / sqrt(Q))   [B, H, N]
    rowsum  = attn.sum(-1)                     == 1 identically (softmax rows)
    outv    = rowsum[:, :, None] * values.mean(0)   -> tile(vmean, H)  [B, H*V]
    out     = outv @ Wo.T + x

Since softmax rows sum to exactly 1 (up to fp rounding ~1e-6, far below the
fp32 output tolerance), the network reduces to a rank-1 correction:

    out[b, i] = x[b, i] + w[i]
    w[i]      = sum_c WoSum[i, c] * vmean[c],  WoSum[i, c] = sum_h Wo[i, h*V + c]

keys / Wq / the softmax drop out entirely. The kernel computes vmean, w and
the broadcast add fully on-device.

Sharding (8 cores, column-parallel over the output feature dim):
  core k owns output columns [256k, 256k+256):
    x_shard  = x[:, 256k:256k+256]      [2048, 256]
    wo_shard = Wo[256k:256k+256, :]     [256, 2048]
    values   = replicated               [8192, 128]
  gather: concatenate core outputs along axis 1.
"""

import numpy as np

B, D, H, Q, N, V = 2048, 2048, 16, 128, 8192, 128
NCORES = 8
CSH = D // NCORES   # 256 output columns per core
VCH = 4             # values loaded/reduced in 4 chunks of 1 MB
XCH = 4             # x/out streamed in 4 chunks of 512 rows (512 KB each)

_CACHE = {}


def _build_nc():
    import concourse.tile as tile
    from concourse import bacc, mybir
    from concourse.masks import make_identity

    f32 = mybir.dt.float32
    nc = bacc.Bacc()
    x_d = nc.declare_dram_parameter("x", [B, CSH], f32, isOutput=False)
    v_d = nc.declare_dram_parameter("values", [N, V], f32, isOutput=False)
    wo_d = nc.declare_dram_parameter("wo", [CSH, D], f32, isOutput=False)
    out_d = nc.declare_dram_parameter("out", [B, CSH], f32, isOutput=True)

    rows_per_vchunk = N // VCH                    # 2048 rows
    a_per_part = rows_per_vchunk // 128           # 16 rows per partition
    vfree = a_per_part * V                        # 2048 floats, contiguous
    t_per_xchunk = B // XCH // 128                # 4 row-tiles per x chunk

    with tile.TileContext(nc) as tc:
        with (
            tc.tile_pool(name="consts", bufs=1) as consts,
            tc.tile_pool(name="vals", bufs=1) as vals,
            tc.tile_pool(name="wop", bufs=1) as wop,
            tc.tile_pool(name="xs", bufs=1) as xs,
            tc.tile_pool(name="small", bufs=1) as small,
            tc.tile_pool(name="ps", bufs=1, space="PSUM") as ps,
        ):
            ones = consts.tile([128, 128], f32, tag="ones")
            nc.vector.memset(ones, 1.0)
            # identity for PE transpose; produced on gpsimd, then copied
            # through DVE so PE matmuls only ever wait on one engine
            # (walrus rejects PE instructions with too many sync waits)
            ident_g = consts.tile([128, 128], f32, tag="ident_g")
            make_identity(nc, ident_g)
            ident = consts.tile([128, 128], f32, tag="ident")
            nc.vector.tensor_copy(ident, ident_g)

            # ---- vmean: sum values over N in three stages ----
            # chunk q, partition p holds 2048 contiguous floats
            # (rows q*2048 + 16p .. +16, all 128 cols)
            vflat = v_d.reshape([VCH, 128, vfree])
            partial4 = small.tile([128, VCH, V], f32, tag="partial4")
            for q in range(VCH):
                vq = vals.tile([128, vfree], f32, tag=f"vq{q}")
                nc.sync.dma_start(out=vq, in_=vflat[q])
                # in-partition: sum the 16 local rows for each of 128 cols
                nc.vector.reduce_sum(
                    out=partial4[:, q, :],
                    in_=vq.rearrange("p (a v) -> p v a", v=V),
                    axis=mybir.AxisListType.X,
                )
            vpart = small.tile([128, V], f32, tag="vpart")
            nc.vector.reduce_sum(
                out=vpart,
                in_=partial4.rearrange("p q v -> p v q"),
                axis=mybir.AxisListType.X,
            )
            # cross-partition sum + column-replicate: psum1[c, m] = vsum[c]
            psum1 = ps.tile([128, 128], f32, tag="psum1")
            nc.tensor.matmul(psum1, lhsT=vpart, rhs=ones, start=True, stop=True)
            vmean_cb = small.tile([128, 128], f32, tag="vmean_cb")
            nc.scalar.activation(
                vmean_cb, psum1, mybir.ActivationFunctionType.Copy, scale=1.0 / N
            )

            # ---- WoSum^T: wsumT[c, i] = sum_h Wo[i, h*V + c] ----
            wflat = wo_d.reshape([CSH // 128, 128, D])
            wsumT = small.tile([128, CSH], f32, tag="wsumT")
            psumT = ps.tile([128, CSH], f32, tag="psumT")
            for t in range(CSH // 128):
                wo_t = wop.tile([128, D], f32, tag=f"wo{t}")
                nc.sync.dma_start(out=wo_t, in_=wflat[t])
                wsum_t = small.tile([128, V], f32, tag=f"wsum{t}")
                nc.vector.reduce_sum(
                    out=wsum_t,
                    in_=wo_t.rearrange("p (h c) -> p c h", c=V),
                    axis=mybir.AxisListType.X,
                )
                nc.tensor.transpose(
                    psumT[:, t * 128 : (t + 1) * 128], wsum_t, ident
                )
            # single evac so the next matmul's rhs has one producer
            nc.scalar.copy(out=wsumT, in_=psumT)

            # ---- w, replicated across partitions: psw[m, i] = w[i] ----
            psw = ps.tile([128, CSH], f32, tag="psw")
            nc.tensor.matmul(psw, lhsT=vmean_cb, rhs=wsumT, start=True, stop=True)
            # evac on DVE (not ACT): the adds below then depend on w_b via
            # same-engine program order, keeping them at one sync wait each
            # (walrus TT encoding rejects 2 waits + 1 update)
            w_b = small.tile([128, CSH], f32, tag="w_b")
            nc.vector.tensor_copy(w_b, psw)

            # ---- out = x + w (w broadcast along rows) ----
            xv = x_d.rearrange("(j t p) c -> j p t c", t=t_per_xchunk, p=128)
            ov = out_d.rearrange("(j t p) c -> j p t c", t=t_per_xchunk, p=128)
            for j in range(XCH):
                xt = xs.tile([128, t_per_xchunk, CSH], f32, tag=f"x{j}")
                nc.sync.dma_start(out=xt, in_=xv[j])
                nc.vector.tensor_add(
                    xt, xt, w_b[:, None, :].broadcast_to([128, t_per_xchunk, CSH])
                )
                nc.sync.dma_start(out=ov[j], in_=xt)
    nc.compile()  # bacc passes: split multi-wait sync (TRN2 allows 1/inst), DCE
    return nc


def _get_nc():
    if "nc" not in _CACHE:
        _CACHE["nc"] = _build_nc()
    return _CACHE["nc"]


def _run(x, values, Wo, trace=False):
    from concourse.bass_utils import run_bass_kernel_spmd

    nc = _get_nc()
    in_maps = []
    for k in range(NCORES):
        sl = slice(k * CSH, (k + 1) * CSH)
        in_maps.append(
            {
                "x": np.ascontiguousarray(x[:, sl]),
                "values": values,
                "wo": np.ascontiguousarray(Wo[sl, :]),
            }
        )
    res = run_bass_kernel_spmd(nc, in_maps, core_ids=list(range(NCORES)), trace=trace)
    out = np.concatenate([res.results[k]["out"] for k in range(NCORES)], axis=1)
    return np.asarray(out, dtype=np.float32), res


def kernel(**inputs) -> np.ndarray:
    x = np.asarray(inputs["x"], dtype=np.float32)
    values = np.asarray(inputs["values"], dtype=np.float32)
    Wo = np.asarray(inputs["Wo"], dtype=np.float32)
    out, _ = _run(x, values, Wo, trace=False)
    return out


# revision 8
# speedup vs baseline: 1.0236x; 1.0236x over previous
"""Trainium2 Bass kernel for nn_MemLayer_7275674600019 (retrieval_knn).

Math: the reference computes
    queries = (x @ Wq.T)                       [B, H, Q]
    attn    = softmax(queries @ keys.T / sqrt(Q))   [B, H, N]
    rowsum  = attn.sum(-1)                     == 1 identically (softmax rows)
    outv    = rowsum[:, :, None] * values.mean(0)   -> tile(vmean, H)  [B, H*V]
    out     = outv @ Wo.T + x

Since softmax rows sum to exactly 1 (up to fp rounding ~1e-6, far below the
fp32 output tolerance), the network reduces to a rank-1 correction:

    out[b, i] = x[b, i] + w[i]
    w[i]      = sum_c WoSum[i, c] * vmean[c],  WoSum[i, c] = sum_h Wo[i, h*V + c]

keys / Wq / the softmax drop out entirely. The kernel computes vmean, w and
the broadcast add fully on-device.

Sharding (8 cores, column-parallel over the output feature dim):
  core k owns output columns [256k, 256k+256):
    x_shard  = x[:, 256k:256k+256]      [2048, 256]
    wo_shard = Wo[256k:256k+256, :]     [256, 2048]
    values   = replicated               [8192, 128]
  gather: concatenate core outputs along axis 1.

Implementation notes:
  - All reductions are unit-stride in-place halving adds on DVE (strided
    tensor_reduce runs ~0.4 elem/cycle; unit-stride TT hits the 2x fp32 mode).
  - Every DMA moves >=1MB with >=8KB contiguous runs per partition
    (x/out are remapped so each partition holds 8 consecutive rows).
  - Cross-partition sum + partition-replication of vmean/w go through two
    tiny PE matmuls and two PE transposes.
  - TRN2 allows 1 sync wait per instruction; bacc's compile() splits the
    rest, but the dataflow is arranged so hot instructions need only one.
"""

import numpy as np

B, D, H, Q, N, V = 2048, 2048, 16, 128, 8192, 128
NCORES = 8
CSH = D // NCORES   # 256 output columns per core
VCH = 4             # values loaded/reduced in 4 chunks of 1 MB
XCH = 2             # x/out streamed in 2 chunks of 1 MB

_CACHE = {}


def _build_nc():
    import concourse.tile as tile
    from concourse import bacc, mybir
    from concourse.masks import make_identity

    f32 = mybir.dt.float32
    nc = bacc.Bacc()
    x_d = nc.declare_dram_parameter("x", [B, CSH], f32, isOutput=False)
    v_d = nc.declare_dram_parameter("values", [N, V], f32, isOutput=False)
    wo_d = nc.declare_dram_parameter("wo", [CSH, D], f32, isOutput=False)
    out_d = nc.declare_dram_parameter("out", [B, CSH], f32, isOutput=True)

    vfree = N // VCH // 128 * V          # 2048 floats/partition per chunk
    xfree = B // XCH // 128 * CSH        # 2048 floats/partition per chunk

    def halve_to_128(t, width):
        # in-place pairwise sum over the outer repeat dim: [p, k*128] -> [p, 128]
        while width > V:
            width //= 2
            nc.vector.tensor_add(t[:, :width], t[:, :width], t[:, width : 2 * width])

    with tile.TileContext(nc) as tc:
        with (
            tc.tile_pool(name="consts", bufs=1) as consts,
            tc.tile_pool(name="vals", bufs=1) as vals,
            tc.tile_pool(name="wop", bufs=1) as wop,
            tc.tile_pool(name="xs", bufs=1) as xs,
            tc.tile_pool(name="small", bufs=1) as small,
            tc.tile_pool(name="ps", bufs=1, space="PSUM") as ps,
        ):
            ones = consts.tile([128, 128], f32, tag="ones")
            nc.vector.memset(ones, 1.0)
            # identity for PE transpose; produced on gpsimd, then copied
            # through DVE so PE matmuls only ever wait on one engine
            ident_g = consts.tile([128, 128], f32, tag="ident_g")
            make_identity(nc, ident_g)
            ident = consts.tile([128, 128], f32, tag="ident")
            nc.vector.tensor_copy(ident, ident_g)

            # ---- vmean: sum values over N ----
            # chunk q, partition p holds rows [q*2048+16p, +16) = 8KB contiguous
            vflat = v_d.reshape([VCH, 128, vfree])
            vq = []
            for q in range(VCH):
                t = vals.tile([128, vfree], f32, tag=f"vq{q}")
                nc.sync.dma_start(out=t, in_=vflat[q])
                halve_to_128(t, vfree)  # chunk sum in t[:, :128]
                vq.append(t)
            v01 = small.tile([128, V], f32, tag="v01")
            v23 = small.tile([128, V], f32, tag="v23")
            vpart = small.tile([128, V], f32, tag="vpart")
            nc.vector.tensor_add(v01, vq[0][:, :V], vq[1][:, :V])
            nc.vector.tensor_add(v23, vq[2][:, :V], vq[3][:, :V])
            nc.vector.tensor_add(vpart, v01, v23)
            # cross-partition sum + column-replicate: psum1[c, m] = vsum[c]
            psum1 = ps.tile([128, 128], f32, tag="psum1")
            nc.tensor.matmul(psum1, lhsT=vpart, rhs=ones, start=True, stop=True)
            vmean_cb = small.tile([128, 128], f32, tag="vmean_cb")
            nc.scalar.activation(
                vmean_cb, psum1, mybir.ActivationFunctionType.Copy, scale=1.0 / N
            )

            # ---- WoSum^T: wsumT[c, i] = sum_h Wo[i, h*V + c] ----
            wflat = wo_d.reshape([CSH // 128, 128, D])
            wsumT = small.tile([128, CSH], f32, tag="wsumT")
            psumT = ps.tile([128, CSH], f32, tag="psumT")
            for t in range(CSH // 128):
                wo_t = wop.tile([128, D], f32, tag=f"wo{t}")
                nc.sync.dma_start(out=wo_t, in_=wflat[t])
                halve_to_128(wo_t, D)  # WoSum tile in wo_t[:, :128]
                nc.tensor.transpose(
                    psumT[:, t * 128 : (t + 1) * 128], wo_t[:, :V], ident
                )
            # single evac so the next matmul's rhs has one producer
            nc.scalar.copy(out=wsumT, in_=psumT)

            # ---- w, replicated across partitions: psw[m, i] = w[i] ----
            psw = ps.tile([128, CSH], f32, tag="psw")
            nc.tensor.matmul(psw, lhsT=vmean_cb, rhs=wsumT, start=True, stop=True)
            # w_rep = w tiled 8x along free dim, built by doubling on DVE
            # (DVE producer => the adds below wait only on their x DMA)
            w_rep = small.tile([128, xfree], f32, tag="w_rep")
            nc.vector.tensor_copy(w_rep[:, :CSH], psw)
            width = CSH
            while width < xfree:
                nc.vector.tensor_copy(
                    w_rep[:, width : 2 * width], w_rep[:, :width]
                )
                width *= 2

            # ---- out = x + w ----
            # chunk j, partition p holds rows [j*1024+8p, +8) = 8KB contiguous
            xv = x_d.reshape([XCH, 128, xfree])
            ov = out_d.reshape([XCH, 128, xfree])
            for j in range(XCH):
                xt = xs.tile([128, xfree], f32, tag=f"x{j}")
                nc.sync.dma_start(out=xt, in_=xv[j])
                nc.vector.tensor_add(xt, xt, w_rep)
                nc.sync.dma_start(out=ov[j], in_=xt)
    nc.compile()  # bacc passes: split multi-wait sync (TRN2 allows 1/inst), DCE
    return nc


def _get_nc():
    if "nc" not in _CACHE:
        _CACHE["nc"] = _build_nc()
    return _CACHE["nc"]


def _run(x, values, Wo, trace=False):
    from concourse.bass_utils import run_bass_kernel_spmd

    nc = _get_nc()
    in_maps = []
    for k in range(NCORES):
        sl = slice(k * CSH, (k + 1) * CSH)
        in_maps.append(
            {
                "x": np.ascontiguousarray(x[:, sl]),
                "values": values,
                "wo": np.ascontiguousarray(Wo[sl, :]),
            }
        )
    res = run_bass_kernel_spmd(nc, in_maps, core_ids=list(range(NCORES)), trace=trace)
    out = np.concatenate([res.results[k]["out"] for k in range(NCORES)], axis=1)
    return np.asarray(out, dtype=np.float32), res


def kernel(**inputs) -> np.ndarray:
    x = np.asarray(inputs["x"], dtype=np.float32)
    values = np.asarray(inputs["values"], dtype=np.float32)
    Wo = np.asarray(inputs["Wo"], dtype=np.float32)
    out, _ = _run(x, values, Wo, trace=False)
    return out


# revision 11
# speedup vs baseline: 1.1594x; 1.1326x over previous
"""Trainium2 Bass kernel for nn_MemLayer_7275674600019 (retrieval_knn).

Math: the reference computes
    queries = (x @ Wq.T)                       [B, H, Q]
    attn    = softmax(queries @ keys.T / sqrt(Q))   [B, H, N]
    rowsum  = attn.sum(-1)                     == 1 identically (softmax rows)
    outv    = rowsum[:, :, None] * values.mean(0)   -> tile(vmean, H)  [B, H*V]
    out     = outv @ Wo.T + x

Since softmax rows sum to exactly 1 (up to fp rounding ~1e-6, far below the
fp32 output tolerance), the network reduces to a rank-1 correction:

    out[b, i] = x[b, i] + w[i]
    w[i]      = sum_c WoSum[i, c] * vmean[c],  WoSum[i, c] = sum_h Wo[i, h*V + c]

keys / Wq / the softmax drop out entirely. The kernel computes vmean, w and
the broadcast add fully on-device.

Sharding (8 cores, column-parallel over the output feature dim):
  core k owns output columns [256k, 256k+256):
    x_shard  = x[:, 256k:256k+256]      [2048, 256]
    wo_shard = Wo[256k:256k+256, :]     [256, 2048]
    values   = replicated               [8192, 128]
  gather: concatenate core outputs along axis 1.

Implementation notes:
  - All reductions are unit-stride in-place halving adds on DVE (strided
    tensor_reduce runs ~0.4 elem/cycle; unit-stride TT hits the 2x fp32 mode).
  - Every DMA moves >=1MB with >=8KB contiguous runs per partition
    (x/out are remapped so each partition holds 8 consecutive rows).
  - Cross-partition sum + partition-replication of vmean/w go through two
    tiny PE matmuls and two PE transposes.
  - TRN2 allows 1 sync wait per instruction; bacc's compile() splits the
    rest, but the dataflow is arranged so hot instructions need only one.
"""

import numpy as np

B, D, H, Q, N, V = 2048, 2048, 16, 128, 8192, 128
NCORES = 8
CSH = D // NCORES   # 256 output columns per core
VCH = 4             # values loaded/reduced in 4 chunks of 1 MB
XCH = 4             # x/out streamed in 4 chunks of 512 KB

_CACHE = {}


def _build_nc():
    import concourse.tile as tile
    from concourse import bacc, mybir
    from concourse.bass import _add_dep_helper
    from concourse.masks import make_identity

    f32 = mybir.dt.float32
    nc = bacc.Bacc()
    x_d = nc.declare_dram_parameter("x", [B, CSH], f32, isOutput=False)
    v_d = nc.declare_dram_parameter("values", [N, V], f32, isOutput=False)
    wo_d = nc.declare_dram_parameter("wo", [CSH, D], f32, isOutput=False)
    out_d = nc.declare_dram_parameter("out", [B, CSH], f32, isOutput=True)

    vfree = N // VCH // 128 * V          # 2048 floats/partition per chunk
    xfree = B // XCH // 128 * CSH        # 2048 floats/partition per chunk

    def halve_to_128(t, width):
        # in-place pairwise sum over the outer repeat dim: [p, k*128] -> [p, 128]
        while width > V:
            width //= 2
            nc.vector.tensor_add(t[:, :width], t[:, :width], t[:, width : 2 * width])

    with tile.TileContext(nc) as tc:
        with (
            tc.tile_pool(name="consts", bufs=1) as consts,
            tc.tile_pool(name="vals", bufs=1) as vals,
            tc.tile_pool(name="wop", bufs=1) as wop,
            tc.tile_pool(name="xs", bufs=1) as xs,
            tc.tile_pool(name="small", bufs=1) as small,
            tc.tile_pool(name="ps", bufs=1, space="PSUM") as ps,
        ):
            ones = consts.tile([128, 128], f32, tag="ones")
            nc.vector.memset(ones, 1.0)
            # identity for PE transpose; produced on gpsimd, then copied
            # through DVE so PE matmuls only ever wait on one engine
            ident_g = consts.tile([128, 128], f32, tag="ident_g")
            make_identity(nc, ident_g)
            ident = consts.tile([128, 128], f32, tag="ident")
            nc.vector.tensor_copy(ident, ident_g)

            # ---- Wo DMAs first, then values; x is held back (see below) so
            # the SDMA round-robin gives w's inputs the full HBM bandwidth ----
            wflat = wo_d.reshape([CSH // 128, 128, D])
            wo_t, wo_dma = [], []
            for t in range(CSH // 128):
                wt = wop.tile([128, D], f32, tag=f"wo{t}")
                wo_dma.append(nc.sync.dma_start(out=wt, in_=wflat[t]))
                wo_t.append(wt)

            # chunk q, partition p holds rows [q*2048+16p, +16) = 8KB contiguous
            vflat = v_d.reshape([VCH, 128, vfree])
            vq, vq_dma = [], []
            for q in range(VCH):
                t = vals.tile([128, vfree], f32, tag=f"vq{q}")
                vq_dma.append(nc.sync.dma_start(out=t, in_=vflat[q]))
                halve_to_128(t, vfree)  # chunk sum in t[:, :128]
                vq.append(t)

            # ---- WoSum^T: wsumT[c, i] = sum_h Wo[i, h*V + c] ----
            wsumT = small.tile([128, CSH], f32, tag="wsumT")
            psumT = ps.tile([128, CSH], f32, tag="psumT")
            for t in range(CSH // 128):
                halve_to_128(wo_t[t], D)  # WoSum tile in wo_t[:, :128]
                nc.tensor.transpose(
                    psumT[:, t * 128 : (t + 1) * 128], wo_t[t][:, :V], ident
                )
            # single evac so the next matmul's rhs has one producer
            nc.scalar.copy(out=wsumT, in_=psumT)

            # ---- vmean ----
            v01 = small.tile([128, V], f32, tag="v01")
            v23 = small.tile([128, V], f32, tag="v23")
            vpart = small.tile([128, V], f32, tag="vpart")
            nc.vector.tensor_add(v01, vq[0][:, :V], vq[1][:, :V])
            nc.vector.tensor_add(v23, vq[2][:, :V], vq[3][:, :V])
            nc.vector.tensor_add(vpart, v01, v23)
            # cross-partition sum + column-replicate: psum1[c, m] = vsum[c]
            psum1 = ps.tile([128, 128], f32, tag="psum1")
            nc.tensor.matmul(psum1, lhsT=vpart, rhs=ones, start=True, stop=True)
            vmean_cb = small.tile([128, 128], f32, tag="vmean_cb")
            nc.scalar.activation(
                vmean_cb, psum1, mybir.ActivationFunctionType.Copy, scale=1.0 / N
            )

            # ---- w, replicated across partitions: psw[m, i] = w[i] ----
            psw = ps.tile([128, CSH], f32, tag="psw")
            nc.tensor.matmul(psw, lhsT=vmean_cb, rhs=wsumT, start=True, stop=True)
            # w_rep = w tiled along free dim, built by doubling on DVE
            # (DVE producer => the adds below wait only on their x DMA)
            w_rep = small.tile([128, xfree], f32, tag="w_rep")
            nc.vector.tensor_copy(w_rep[:, :CSH], psw)
            width = CSH
            while width < xfree:
                nc.vector.tensor_copy(
                    w_rep[:, width : 2 * width], w_rep[:, :width]
                )
                width *= 2

            # ---- out = x + w ----
            # chunk j, partition p holds 4 consecutive rows = 4KB contiguous
            xv = x_d.reshape([XCH, 128, xfree])
            ov = out_d.reshape([XCH, 128, xfree])
            for j in range(XCH):
                xt = xs.tile([128, xfree], f32, tag=f"x{j}")
                xd = nc.sync.dma_start(out=xt, in_=xv[j])
                # hold x back until the last values chunk has drained so the
                # w-input stream isn't time-shared with x under round-robin
                _add_dep_helper(
                    xd.ins, vq_dma[-1].ins, sync=True,
                    reason="prioritize values stream over x",
                )
                nc.vector.tensor_add(xt, xt, w_rep)
                nc.sync.dma_start(out=ov[j], in_=xt)
    nc.compile()  # bacc passes: split multi-wait sync (TRN2 allows 1/inst), DCE
    return nc


def _get_nc():
    if "nc" not in _CACHE:
        _CACHE["nc"] = _build_nc()
    return _CACHE["nc"]


def _run(x, values, Wo, trace=False):
    from concourse.bass_utils import run_bass_kernel_spmd

    nc = _get_nc()
    in_maps = []
    for k in range(NCORES):
        sl = slice(k * CSH, (k + 1) * CSH)
        in_maps.append(
            {
                "x": np.ascontiguousarray(x[:, sl]),
                "values": values,
                "wo": np.ascontiguousarray(Wo[sl, :]),
            }
        )
    res = run_bass_kernel_spmd(nc, in_maps, core_ids=list(range(NCORES)), trace=trace)
    out = np.concatenate([res.results[k]["out"] for k in range(NCORES)], axis=1)
    return np.asarray(out, dtype=np.float32), res


def kernel(**inputs) -> np.ndarray:
    x = np.asarray(inputs["x"], dtype=np.float32)
    values = np.asarray(inputs["values"], dtype=np.float32)
    Wo = np.asarray(inputs["Wo"], dtype=np.float32)
    out, _ = _run(x, values, Wo, trace=False)
    return out


# revision 13
# speedup vs baseline: 1.2124x; 1.0458x over previous
"""Trainium2 Bass kernel for nn_MemLayer_7275674600019 (retrieval_knn).

Math: the reference computes
    queries = (x @ Wq.T)                       [B, H, Q]
    attn    = softmax(queries @ keys.T / sqrt(Q))   [B, H, N]
    rowsum  = attn.sum(-1)                     == 1 identically (softmax rows)
    outv    = rowsum[:, :, None] * values.mean(0)   -> tile(vmean, H)  [B, H*V]
    out     = outv @ Wo.T + x

Since softmax rows sum to exactly 1 (up to fp rounding ~1e-6, far below the
fp32 output tolerance), the network reduces to a rank-1 correction:

    out[b, i] = x[b, i] + w[i]
    w[i]      = sum_c WoSum[i, c] * vmean[c],  WoSum[i, c] = sum_h Wo[i, h*V + c]

keys / Wq / the softmax drop out entirely. The kernel computes vmean, w and
the broadcast add fully on-device.

Sharding (8 cores, column-parallel over the output feature dim):
  core k owns output columns [256k, 256k+256):
    x_shard  = x[:, 256k:256k+256]      [2048, 256]
    wo_shard = Wo[256k:256k+256, :]     [256, 2048]
    values   = replicated               [8192, 128]
  gather: concatenate core outputs along axis 1.

Implementation notes:
  - All reductions are unit-stride in-place halving adds on DVE (strided
    tensor_reduce runs ~0.4 elem/cycle; unit-stride TT hits the 2x fp32 mode).
  - Every DMA moves >=1MB with >=8KB contiguous runs per partition
    (x/out are remapped so each partition holds 8 consecutive rows).
  - Cross-partition sum + partition-replication of vmean/w go through two
    tiny PE matmuls and two PE transposes.
  - TRN2 allows 1 sync wait per instruction; bacc's compile() splits the
    rest, but the dataflow is arranged so hot instructions need only one.
"""

import numpy as np

B, D, H, Q, N, V = 2048, 2048, 16, 128, 8192, 128
NCORES = 8
CSH = D // NCORES   # 256 output columns per core
VCH = 4             # values loaded/reduced in 4 chunks of 1 MB
XCH = 4             # x/out streamed in 4 chunks of 512 KB

_CACHE = {}


def _build_nc():
    import concourse.tile as tile
    from concourse import bacc, mybir
    from concourse.bass import _add_dep_helper
    from concourse.masks import make_identity

    f32 = mybir.dt.float32
    nc = bacc.Bacc()
    x_d = nc.declare_dram_parameter("x", [B, CSH], f32, isOutput=False)
    v_d = nc.declare_dram_parameter("values", [N, V], f32, isOutput=False)
    wo_d = nc.declare_dram_parameter("wo", [CSH, D], f32, isOutput=False)
    out_d = nc.declare_dram_parameter("out", [B, CSH], f32, isOutput=True)

    vfree = N // VCH // 128 * V          # 2048 floats/partition per chunk
    xfree = B // XCH // 128 * CSH        # 2048 floats/partition per chunk

    def halve_to_128(t, width):
        # in-place pairwise sum over the outer repeat dim: [p, k*128] -> [p, 128]
        while width > V:
            width //= 2
            nc.vector.tensor_add(t[:, :width], t[:, :width], t[:, width : 2 * width])

    with tile.TileContext(nc) as tc:
        with (
            tc.tile_pool(name="consts", bufs=1) as consts,
            tc.tile_pool(name="vals", bufs=1) as vals,
            tc.tile_pool(name="wop", bufs=1) as wop,
            tc.tile_pool(name="xs", bufs=1) as xs,
            tc.tile_pool(name="small", bufs=1) as small,
            tc.tile_pool(name="ps", bufs=1, space="PSUM") as ps,
        ):
            ones = consts.tile([128, 128], f32, tag="ones")
            nc.vector.memset(ones, 1.0)
            # identity for PE transpose; produced on gpsimd, then copied
            # through DVE so PE matmuls only ever wait on one engine
            ident_g = consts.tile([128, 128], f32, tag="ident_g")
            make_identity(nc, ident_g)
            ident = consts.tile([128, 128], f32, tag="ident")
            nc.vector.tensor_copy(ident, ident_g)

            # ---- Wo DMAs first, then values; x is held back (see below) so
            # the SDMA round-robin gives w's inputs the full HBM bandwidth ----
            wflat = wo_d.reshape([CSH // 128, 128, D])
            wo_t, wo_dma = [], []
            for t in range(CSH // 128):
                wt = wop.tile([128, D], f32, tag=f"wo{t}")
                wo_dma.append(nc.sync.dma_start(out=wt, in_=wflat[t]))
                wo_t.append(wt)

            # chunk q, partition p holds rows [q*2048+16p, +16) = 8KB contiguous
            # each chunk's [128, V] partial feeds a PSUM-accumulating matmul
            # (lhsT=partial, rhs=ones): psum1[c, m] = sum_p sum_q partial[p, c]
            # -- the cross-partition total, already column-replicated for M2.
            vflat = v_d.reshape([VCH, 128, vfree])
            psum1 = ps.tile([128, 128], f32, tag="psum1")
            vq, vq_dma = [], []
            for q in range(VCH):
                t = vals.tile([128, vfree], f32, tag=f"vq{q}")
                vq_dma.append(nc.sync.dma_start(out=t, in_=vflat[q]))
                halve_to_128(t, vfree)  # chunk sum in t[:, :128]
                nc.tensor.matmul(
                    psum1,
                    lhsT=t[:, :V],
                    rhs=ones,
                    start=(q == 0),
                    stop=(q == VCH - 1),
                    skip_group_check=True,
                )
                vq.append(t)

            # ---- WoSum^T: wsumT[c, i] = sum_h Wo[i, h*V + c] ----
            wsumT = small.tile([128, CSH], f32, tag="wsumT")
            psumT = ps.tile([128, CSH], f32, tag="psumT")
            for t in range(CSH // 128):
                halve_to_128(wo_t[t], D)  # WoSum tile in wo_t[:, :128]
                nc.tensor.transpose(
                    psumT[:, t * 128 : (t + 1) * 128], wo_t[t][:, :V], ident
                )
            # single evac so the next matmul's rhs has one producer
            nc.scalar.copy(out=wsumT, in_=psumT)

            # ---- vmean (column-replicated), scaled during PSUM evac ----
            vmean_cb = small.tile([128, 128], f32, tag="vmean_cb")
            nc.scalar.activation(
                vmean_cb, psum1, mybir.ActivationFunctionType.Copy, scale=1.0 / N
            )

            # ---- w, replicated across partitions: psw[m, i] = w[i] ----
            psw = ps.tile([128, CSH], f32, tag="psw")
            nc.tensor.matmul(psw, lhsT=vmean_cb, rhs=wsumT, start=True, stop=True)

            # ---- out = x + w ----
            # chunk j, partition p holds 4 consecutive rows = 4KB contiguous
            xv = x_d.reshape([XCH, 128, xfree])
            ov = out_d.reshape([XCH, 128, xfree])
            for j in range(XCH):
                xt = xs.tile([128, xfree], f32, tag=f"x{j}")
                xd = nc.sync.dma_start(out=xt, in_=xv[j])
                # hold x back until the last values chunk has drained so the
                # w-input stream isn't time-shared with x under round-robin
                _add_dep_helper(
                    xd.ins, vq_dma[-1].ins, sync=True,
                    reason="prioritize values stream over x",
                )
                xt3 = xt.rearrange("p (r c) -> p r c", c=CSH)
                nc.vector.tensor_add(
                    xt3,
                    xt3,
                    psw[:, None, :].broadcast_to([128, xfree // CSH, CSH]),
                )
                nc.sync.dma_start(out=ov[j], in_=xt)
    nc.compile()  # bacc passes: split multi-wait sync (TRN2 allows 1/inst), DCE
    return nc


def _get_nc():
    if "nc" not in _CACHE:
        _CACHE["nc"] = _build_nc()
    return _CACHE["nc"]


def _run(x, values, Wo, trace=False):
    from concourse.bass_utils import run_bass_kernel_spmd

    nc = _get_nc()
    in_maps = []
    for k in range(NCORES):
        sl = slice(k * CSH, (k + 1) * CSH)
        in_maps.append(
            {
                "x": np.ascontiguousarray(x[:, sl]),
                "values": values,
                "wo": np.ascontiguousarray(Wo[sl, :]),
            }
        )
    res = run_bass_kernel_spmd(nc, in_maps, core_ids=list(range(NCORES)), trace=trace)
    out = np.concatenate([res.results[k]["out"] for k in range(NCORES)], axis=1)
    return np.asarray(out, dtype=np.float32), res


def kernel(**inputs) -> np.ndarray:
    x = np.asarray(inputs["x"], dtype=np.float32)
    values = np.asarray(inputs["values"], dtype=np.float32)
    Wo = np.asarray(inputs["Wo"], dtype=np.float32)
    out, _ = _run(x, values, Wo, trace=False)
    return out


# revision 19
# speedup vs baseline: 1.2155x; 1.0025x over previous
"""Trainium2 Bass kernel for nn_MemLayer_7275674600019 (retrieval_knn).

Math: the reference computes
    queries = (x @ Wq.T)                       [B, H, Q]
    attn    = softmax(queries @ keys.T / sqrt(Q))   [B, H, N]
    rowsum  = attn.sum(-1)                     == 1 identically (softmax rows)
    outv    = rowsum[:, :, None] * values.mean(0)   -> tile(vmean, H)  [B, H*V]
    out     = outv @ Wo.T + x

Since softmax rows sum to exactly 1 (up to fp rounding ~1e-6, far below the
fp32 output tolerance), the network reduces to a rank-1 correction:

    out[b, i] = x[b, i] + w[i]
    w[i]      = sum_c WoSum[i, c] * vmean[c],  WoSum[i, c] = sum_h Wo[i, h*V + c]

keys / Wq / the softmax drop out entirely. The kernel computes vmean, w and
the broadcast add fully on-device.

Sharding (8 cores, column-parallel over the output feature dim):
  core k owns output columns [256k, 256k+256):
    x_shard  = x[:, 256k:256k+256]      [2048, 256]
    wo_shard = Wo[256k:256k+256, :]     [256, 2048]
    values   = replicated               [8192, 128]
  gather: concatenate core outputs along axis 1.

Implementation notes:
  - All reductions are unit-stride in-place halving adds on DVE (strided
    tensor_reduce runs ~0.4 elem/cycle; unit-stride TT hits the 2x fp32 mode).
  - Every DMA moves >=1MB with >=8KB contiguous runs per partition
    (x/out are remapped so each partition holds 8 consecutive rows).
  - Cross-partition sum + partition-replication of vmean/w go through two
    tiny PE matmuls and two PE transposes.
  - TRN2 allows 1 sync wait per instruction; bacc's compile() splits the
    rest, but the dataflow is arranged so hot instructions need only one.
"""

import numpy as np

B, D, H, Q, N, V = 2048, 2048, 16, 128, 8192, 128
NCORES = 8
CSH = D // NCORES   # 256 output columns per core
# values chunk widths (floats per partition; width w <=> w rows, w/256 MB).
# Geometric tail: the last chunk is small so its post-DMA halving+matmul
# chain off the critical path is short.
VWIDTHS = [2048, 2048, 2048, 1024, 512, 512]
XCH = 4             # x/out streamed in 4 chunks of 512 KB

_CACHE = {}


def _build_nc():
    import concourse.tile as tile
    from concourse import bacc, mybir
    from concourse.bass import _add_dep_helper
    from concourse.masks import make_identity

    f32 = mybir.dt.float32
    nc = bacc.Bacc()
    x_d = nc.declare_dram_parameter("x", [B, CSH], f32, isOutput=False)
    v_d = nc.declare_dram_parameter("values", [N, V], f32, isOutput=False)
    wo_d = nc.declare_dram_parameter("wo", [CSH, D], f32, isOutput=False)
    out_d = nc.declare_dram_parameter("out", [B, CSH], f32, isOutput=True)

    assert sum(VWIDTHS) * 128 == N * V
    xfree = B // XCH // 128 * CSH        # floats/partition per x chunk

    def halve_to_128(t, width):
        # in-place pairwise sum over the outer repeat dim: [p, k*128] -> [p, 128]
        while width > V:
            width //= 2
            nc.vector.tensor_add(t[:, :width], t[:, :width], t[:, width : 2 * width])

    with tile.TileContext(nc) as tc:
        with (
            tc.tile_pool(name="consts", bufs=1) as consts,
            tc.tile_pool(name="vals", bufs=1) as vals,
            tc.tile_pool(name="wop", bufs=1) as wop,
            tc.tile_pool(name="xs", bufs=1) as xs,
            tc.tile_pool(name="small", bufs=1) as small,
            tc.tile_pool(name="ps", bufs=1, space="PSUM") as ps,
        ):
            ones = consts.tile([128, 128], f32, tag="ones")
            nc.vector.memset(ones, 1.0)
            # identity for PE transpose; produced on gpsimd, then copied
            # through DVE so PE matmuls only ever wait on one engine
            ident_g = consts.tile([128, 128], f32, tag="ident_g")
            make_identity(nc, ident_g)
            ident = consts.tile([128, 128], f32, tag="ident")
            nc.vector.tensor_copy(ident, ident_g)

            # ---- Wo DMAs first, then values; x is held back (see below) so
            # the SDMA round-robin gives w's inputs the full HBM bandwidth ----
            wflat = wo_d.reshape([CSH // 128, 128, D])
            wo_t, wo_dma = [], []
            for t in range(CSH // 128):
                wt = wop.tile([128, D], f32, tag=f"wo{t}")
                wo_dma.append(nc.sync.dma_start(out=wt, in_=wflat[t]))
                wo_t.append(wt)

            # chunk q: partition p holds w consecutive values-floats (>=2KB
            # contiguous). Each chunk's [128, V] partial feeds a
            # PSUM-accumulating matmul (lhsT=partial, rhs=ones):
            # psum1[c, m] = sum_q sum_p partial[p, c] -- the cross-partition
            # total, already column-replicated for M2.
            psum1 = ps.tile([128, 128], f32, tag="psum1")
            vq_dma = []
            off = 0  # in floats/partition across the flat [128, N*V/128] view
            for q, w in enumerate(VWIDTHS):
                t = vals.tile([128, w], f32, tag=f"vq{q}")
                # chunk covers flat floats [off*128, off*128 + 128*w)
                src = v_d.reshape([N * V // w // 128, 128, w])[off // w]
                vq_dma.append(nc.sync.dma_start(out=t, in_=src))
                off += w
                halve_to_128(t, w)  # chunk sum in t[:, :128]
                nc.tensor.matmul(
                    psum1,
                    lhsT=t[:, :V],
                    rhs=ones,
                    start=(q == 0),
                    stop=(q == len(VWIDTHS) - 1),
                    skip_group_check=True,
                )

            # ---- WoSum^T: wsumT[c, i] = sum_h Wo[i, h*V + c] ----
            wsumT = small.tile([128, CSH], f32, tag="wsumT")
            psumT = ps.tile([128, CSH], f32, tag="psumT")
            for t in range(CSH // 128):
                halve_to_128(wo_t[t], D)  # WoSum tile in wo_t[:, :128]
                nc.tensor.transpose(
                    psumT[:, t * 128 : (t + 1) * 128], wo_t[t][:, :V], ident
                )
            # single evac so the next matmul's rhs has one producer
            nc.scalar.copy(out=wsumT, in_=psumT)

            # ---- vmean (column-replicated), scaled during PSUM evac ----
            vmean_cb = small.tile([128, 128], f32, tag="vmean_cb")
            nc.scalar.activation(
                vmean_cb, psum1, mybir.ActivationFunctionType.Copy, scale=1.0 / N
            )

            # ---- w, replicated across partitions: psw[m, i] = w[i] ----
            psw = ps.tile([128, CSH], f32, tag="psw")
            nc.tensor.matmul(psw, lhsT=vmean_cb, rhs=wsumT, start=True, stop=True)
            # SBUF copy of w for the gpsimd adds (gpsimd cannot read PSUM)
            w_sb = small.tile([128, CSH], f32, tag="w_sb")
            nc.scalar.copy(out=w_sb, in_=psw)

            # ---- out = x + w ----
            # chunk j, partition p holds 4 consecutive rows = 4KB contiguous.
            # The last chunk's add runs on gpsimd, in parallel with the DVE
            # adds, so the final store isn't serialized behind all DVE work.
            xv = x_d.reshape([XCH, 128, xfree])
            ov = out_d.reshape([XCH, 128, xfree])
            for j in range(XCH):
                xt = xs.tile([128, xfree], f32, tag=f"x{j}")
                xd = nc.sync.dma_start(out=xt, in_=xv[j])
                # hold x back until the last values chunk has drained so the
                # w-input stream isn't time-shared with x under round-robin
                _add_dep_helper(
                    xd.ins, vq_dma[-1].ins, sync=True,
                    reason="prioritize values stream over x",
                )
                xt3 = xt.rearrange("p (r c) -> p r c", c=CSH)
                wb3 = [128, xfree // CSH, CSH]
                if j == XCH - 1:
                    nc.gpsimd.tensor_add(
                        xt3, xt3, w_sb[:, None, :].broadcast_to(wb3)
                    )
                else:
                    nc.vector.tensor_add(
                        xt3, xt3, psw[:, None, :].broadcast_to(wb3)
                    )
                nc.sync.dma_start(out=ov[j], in_=xt)
    nc.compile()  # bacc passes: split multi-wait sync (TRN2 allows 1/inst), DCE
    return nc


def _get_nc():
    if "nc" not in _CACHE:
        _CACHE["nc"] = _build_nc()
    return _CACHE["nc"]


def _run(x, values, Wo, trace=False):
    from concourse.bass_utils import run_bass_kernel_spmd

    nc = _get_nc()
    in_maps = []
    for k in range(NCORES):
        sl = slice(k * CSH, (k + 1) * CSH)
        in_maps.append(
            {
                "x": np.ascontiguousarray(x[:, sl]),
                "values": values,
                "wo": np.ascontiguousarray(Wo[sl, :]),
            }
        )
    res = run_bass_kernel_spmd(nc, in_maps, core_ids=list(range(NCORES)), trace=trace)
    out = np.concatenate([res.results[k]["out"] for k in range(NCORES)], axis=1)
    return np.asarray(out, dtype=np.float32), res


def kernel(**inputs) -> np.ndarray:
    x = np.asarray(inputs["x"], dtype=np.float32)
    values = np.asarray(inputs["values"], dtype=np.float32)
    Wo = np.asarray(inputs["Wo"], dtype=np.float32)
    out, _ = _run(x, values, Wo, trace=False)
    return out
